# revision 1
# baseline (speedup 1.0000x reference)
"""Trainium2 Bass kernel for nn_Attention_20091857010765.

8 NeuronCores, pure data parallel over batch (B=8 -> 1 batch element per core).

Per-core dataflow (C=128 channels on SBUF partitions, bf16 compute, fp32 PSUM):
  downsampled path (64x64 = 4096 positions):
    q  = depthwise 3x3/s2 conv     -> DVE: 9 fused MAC taps
    k  = dense 3x3/s2 conv         -> PE: 9-tap accumulated matmuls, 512-wide
    a  = pw(dw(x))                 -> PE: 9-tap matmuls with host-fused weights
    l2 norms                       -> ACT Square+accum_out (sum over free dim)
    qT,kT,aT                       -> DMA-transpose 128-col chunks (bf16)
    G1 = q @ aT, G2 = a @ kT       -> PE: 32-chunk accumulated matmuls
    masked softmax (per-head blocks via -30 bias masks, norm scales folded in)
    W = wout @ A1 @ A2             -> PE: two tiny matmuls + one PE transpose
  full-res path (32 tiles x 512 positions):
    v = (wv @ x) * illu            -> PE + DVE
    out_tile = W @ v               -> PE, 8 batched DMAs out

Layout trick: the W axis is stored even-cols-first / odd-cols-second
("split" layout). Stride-2 conv taps then read contiguous runs in the
last dim, which keeps the DVE 2x/4x fast modes for the depthwise conv
(strided reads would force the 1x path). The full-res interior view
stays a contiguous 128-wide window (a permutation of the original
columns); illu is permuted identically on the host and the output is
unpermuted after download, so the elementwise and matmul stages are
oblivious to the ordering.
"""

import sys

sys.path.insert(0, "/opt/trn_rl_repo")

import numpy as np
import ml_dtypes

BF16 = ml_dtypes.bfloat16
TAPS = [(dy, dx) for dy in range(3) for dx in range(3)]
# split-layout column range for each dx: even cols at [0,65), odd at [65,130)
DX_COL = {0: (0, 64), 1: (65, 129), 2: (1, 65)}

B, C, H, W = 8, 128, 128, 128
HP = H + 2  # padded
S_FULL = H * W  # 16384
S_DS = (H // 2) * (W // 2)  # 4096
N_TILE = 512
N_CONV_TILES = S_DS // N_TILE  # 8
N_CHUNKS = S_DS // 128  # 32
NEG = -30.0
# device interior position p (0..127) holds original column WLIST[p]
WLIST = np.r_[1:128:2, 0:127:2]

_CACHE = {}


def _build(reps=1, loop_n=None):
    import concourse.bass as bass
    import concourse.tile as tile
    import concourse.mybir as mybir
    from concourse import bacc
    from concourse.bass import ts
    from contextlib import ExitStack

    dt = mybir.dt
    F32, BF = dt.float32, dt.bfloat16
    AF = mybir.ActivationFunctionType
    OP = mybir.AluOpType

    nc = bacc.Bacc("TRN2", target_bir_lowering=False, debug=False, num_devices=8)

    def din(name, shape, dtyp=BF):
        return nc.dram_tensor(name, shape, dtyp, kind="ExternalInput").ap()

    xpad_d = din("xpad", [C, HP * HP])
    illu_d = din("illu", [C, S_FULL])
    wkT_d = din("wkT", [C, 9 * 128])
    waT_d = din("waT", [C, 9 * 64])
    wvT_d = din("wvT", [C, 128])
    woutT_d = din("woutT", [C, 128])
    wq9_d = din("wq9", [C, 9], F32)
    ta_d = din("ta", [C, 1], F32)
    tv_d = din("tv", [64, 1], F32)
    mask1_d = din("mask1", [C, 64], F32)
    mask2_d = din("mask2", [64, 128], F32)
    ident_d = din("ident", [C, 128], F32)
    ones_d = din("ones", [1, 128])
    out_d = nc.dram_tensor("out", [C, S_FULL], BF, kind="ExternalOutput").ap()

    with tile.TileContext(nc) as tc, ExitStack() as ctx:
        const = ctx.enter_context(tc.tile_pool(name="const", bufs=1))
        big = ctx.enter_context(tc.tile_pool(name="big", bufs=1))
        small = ctx.enter_context(tc.tile_pool(name="small", bufs=2))
        ps_mm = ctx.enter_context(tc.tile_pool(name="psmm", bufs=3, space="PSUM"))
        ps_v = ctx.enter_context(tc.tile_pool(name="psv", bufs=3, space="PSUM"))
        ps_sm = ctx.enter_context(tc.tile_pool(name="pssm", bufs=2, space="PSUM"))

        # ---- const loads ----
        def load_const(name, ap_d, shape, dtyp):
            t = const.tile(shape, dtyp, tag=name, name=f"c_{name}")
            nc.sync.dma_start(t[:], ap_d)
            return t

        wkT = load_const("wkT", wkT_d, [C, 9 * 128], BF)
        waT = load_const("waT", waT_d, [C, 9 * 64], BF)
        wvT = load_const("wvT", wvT_d, [C, 128], BF)
        woutT = load_const("woutT", woutT_d, [C, 128], BF)
        wq9 = load_const("wq9", wq9_d, [C, 9], F32)
        ta = load_const("ta", ta_d, [C, 1], F32)
        tv = load_const("tv", tv_d, [64, 1], F32)
        mask1 = load_const("mask1", mask1_d, [C, 64], F32)
        mask2 = load_const("mask2", mask2_d, [64, 128], F32)
        ident = load_const("ident", ident_d, [C, 128], F32)
        ones = load_const("ones", ones_d, [1, 128], BF)

        wkT3 = wkT[:].rearrange("p (t c) -> p t c", t=9)
        waT3 = waT[:].rearrange("p (t c) -> p t c", t=9)

        import contextlib
        if loop_n is not None:
            rep_ctx = lambda: tc.For_i(0, loop_n, 1)
        else:
            rep_ctx = contextlib.nullcontext
        for _rep in range(reps):
          with rep_ctx():
            # ---- input loads: x in 9 row-aligned chunks, illu on SWDGE ----
            xpad = big.tile([C, HP * HP], BF, tag="xpad")
            x_last = None
            for j in range(9):
                lo = 16 * j * HP
                hi = min(16 * (j + 1), HP) * HP
                eng = nc.sync if j % 2 == 0 else nc.scalar
                x_last = eng.dma_start(xpad[:, lo:hi], xpad_d[:, lo:hi])
            xp3 = xpad[:].rearrange("p (h w) -> p h w", h=HP, w=HP)

            illu_q = [big.tile([C, 4096], BF, tag=f"illu{g}", name=f"illu{g}")
                      for g in range(4)]

            def tap_view(t, r0, nrows):
                dy, dx = TAPS[t]
                c0, c1 = DX_COL[dx]
                return xp3[:, 2 * r0 + dy : 2 * r0 + dy + 2 * nrows : 2, c0:c1]

            # ---- q depthwise conv (DVE): per tap mul at 4x then add at 2x
            #      (the fused scalar_tensor_tensor MAC runs at 1x -- slower) ----
            q_sb = big.tile([C, S_DS], BF, tag="q")
            q3 = q_sb[:].rearrange("p (h w) -> p h w", h=64, w=64)
            qT = big.tile([C, S_DS], BF, tag="qT")
            qT3 = qT[:].rearrange("p (c j) -> p c j", c=N_CHUNKS)
            qtmp = big.tile([C, 1024], BF, tag="qtmp")
            qtmp3 = qtmp[:].rearrange("p (h w) -> p h w", h=16, w=64)
            for qg in range(4):
                r = 16 * qg
                o = q3[:, r : r + 16, :]
                for t in range(9):
                    view = tap_view(t, r, 16)
                    if t == 0:
                        nc.vector.tensor_scalar(o, view, wq9[:, 0:1], None, op0=OP.mult)
                    else:
                        nc.vector.tensor_scalar(
                            qtmp3, view, wq9[:, t : t + 1], None, op0=OP.mult
                        )
                        nc.vector.tensor_tensor(o, o, qtmp3, op=OP.add)
                # one chunked-transpose instruction per quarter (8 chunks)
                nc.sync.dma_start(
                    qT3[:, 8 * qg : 8 * qg + 8, :],
                    q_sb[:, ts(qg, 1024)], transpose=True,
                )

            # ---- conv phase: k, a (PE); one chunked transpose per 1024 cols ----
            k_sb = big.tile([C, S_DS], BF, tag="k")
            a_sb = big.tile([64, S_DS], BF, tag="a")
            kT = big.tile([C, S_DS], BF, tag="kT")
            aT = big.tile([C, N_CHUNKS * 64], BF, tag="aT")
            kT3 = kT[:].rearrange("p (c j) -> p c j", c=N_CHUNKS)
            aT3 = aT[:].rearrange("p (c j) -> p c j", c=N_CHUNKS)
            v_sb = big.tile([C, S_FULL], BF, tag="v")

            for j in range(N_CONV_TILES):
                psk = ps_mm.tile([C, N_TILE], F32, tag="mm")
                for t in range(9):
                    view = tap_view(t, 8 * j, 8)
                    nc.tensor.matmul(
                        psk[:], wkT3[:, t, :], view, start=(t == 0), stop=(t == 8)
                    )
                nc.scalar.copy(k_sb[:, ts(j, N_TILE)], psk[:])
                psa = ps_mm.tile([64, N_TILE], F32, tag="mm")
                for t in range(9):
                    view = tap_view(t, 8 * j, 8)
                    nc.tensor.matmul(
                        psa[:], waT3[:, t, :], view, start=(t == 0), stop=(t == 8)
                    )
                nc.scalar.copy(a_sb[:, ts(j, N_TILE)], psa[:])
                if j % 2 == 1:
                    h = j // 2
                    nc.sync.dma_start(
                        kT3[:, 8 * h : 8 * h + 8, :], k_sb[:, ts(h, 1024)],
                        transpose=True,
                    )
                    nc.sync.dma_start(
                        aT3[:, 8 * h : 8 * h + 8, :], a_sb[:, ts(h, 1024)],
                        transpose=True,
                    )

            from concourse.tile_rust import add_dep_helper
            for g in range(4):
                d = nc.scalar.dma_start(illu_q[g][:], illu_d[:, ts(g, 4096)])
                add_dep_helper(d.ins, x_last.ins, reason="illu yields head BW to x")

            def v_tile(i):
                xv = xp3[:, 4 * i + 1 : 4 * i + 5, 1 : 1 + 128]
                vp = ps_v.tile([C, N_TILE], F32, tag="vmm", name=f"vp{i}")
                nc.tensor.matmul(vp[:], wvT[:], xv, start=True, stop=True)
                nc.vector.tensor_tensor(
                    v_sb[:, ts(i, N_TILE)], vp[:],
                    illu_q[i // 8][:, ts(i % 8, N_TILE)], op=OP.mult,
                )

            # ---- norms: sum of squares along free dim via ACT accum ----
            scr = big.tile([C, N_TILE], BF, tag="scr")
            nq2p = small.tile([C, 4], F32, tag="nq2p")
            nk2p = small.tile([C, N_CONV_TILES], F32, tag="nk2p")
            na2p = small.tile([64, N_CONV_TILES], F32, tag="na2p")
            scrq = big.tile([C, 1024], BF, tag="scrq")
            for j in range(N_CONV_TILES):
                nc.scalar.activation(
                    scr[:], k_sb[:, ts(j, N_TILE)], AF.Square,
                    accum_out=nk2p[:, j : j + 1],
                )
                nc.scalar.activation(
                    scr[:64, :], a_sb[:, ts(j, N_TILE)], AF.Square,
                    accum_out=na2p[:, j : j + 1],
                )
            for qg in range(4):
                nc.scalar.activation(
                    scrq[:], q_sb[:, ts(qg, 1024)], AF.Square,
                    accum_out=nq2p[:, qg : qg + 1],
                )
            nq2 = small.tile([C, 1], F32, tag="nq2")
            nc.vector.tensor_reduce(
                nq2[:], nq2p[:], axis=mybir.AxisListType.X, op=OP.add
            )
            nk2 = small.tile([C, 1], F32, tag="nk2")
            na2 = small.tile([64, 1], F32, tag="na2")
            nc.vector.tensor_reduce(nk2[:], nk2p[:], axis=mybir.AxisListType.X, op=OP.add)
            nc.vector.tensor_reduce(na2[:], na2p[:], axis=mybir.AxisListType.X, op=OP.add)

            # rsqrt = sqrt(1/x); DVE reciprocal is the accurate one
            rq = small.tile([C, 1], F32, tag="rq")
            rk = small.tile([C, 1], F32, tag="rk")
            ra = small.tile([64, 1], F32, tag="ra")
            for n2, r in ((nq2, rq), (nk2, rk), (na2, ra)):
                tmp = small.tile([n2.shape[0], 1], F32, tag="rtmp", name="rtmp")
                nc.vector.reciprocal(tmp[:], n2[:])
                nc.scalar.activation(r[:], tmp[:], AF.Sqrt)

            scale1 = small.tile([C, 1], F32, tag="scale1")
            scale2 = small.tile([64, 1], F32, tag="scale2")
            nc.vector.tensor_tensor(scale1[:], rq[:], ta[:], op=OP.mult)
            nc.vector.tensor_tensor(scale2[:], ra[:], tv[:], op=OP.mult)

            # column-scale broadcast matrices via ones-matmuls
            raT = small.tile([1, 64], BF, tag="raT")
            rkT = small.tile([1, 128], BF, tag="rkT")
            nc.gpsimd.dma_start(raT[:], ra[:])  # cast + reshape (SWDGE)
            nc.gpsimd.dma_start(rkT[:], rk[:])
            rab_ps = ps_sm.tile([C, 64], F32, tag="sm")
            rkb_ps = ps_sm.tile([64, 128], F32, tag="sm")
            nc.tensor.matmul(rab_ps[:], ones[:], raT[:], start=True, stop=True)
            nc.tensor.matmul(rkb_ps[:], ones[:, :64], rkT[:], start=True, stop=True)
            rab = small.tile([C, 64], F32, tag="rab")
            rkb = small.tile([64, 128], F32, tag="rkb")
            nc.vector.tensor_copy(rab[:], rab_ps[:])
            nc.vector.tensor_copy(rkb[:], rkb_ps[:])

            # ---- Gram matrices ----
            g1 = ps_sm.tile([C, 64], F32, tag="sm")
            g2 = ps_sm.tile([64, 128], F32, tag="sm")
            for c in range(N_CHUNKS):
                nc.tensor.matmul(
                    g2[:], aT[:, ts(c, 64)], kT[:, ts(c, 128)],
                    start=(c == 0), stop=(c == N_CHUNKS - 1),
                )
            for c in range(N_CHUNKS):
                nc.tensor.matmul(
                    g1[:], qT[:, ts(c, 128)], aT[:, ts(c, 64)],
                    start=(c == 0), stop=(c == N_CHUNKS - 1),
                )

            # ---- masked softmaxes ----
            def softmax(g_ps, scale_pp, colb, maskb, p_shape, out_dt):
                rows = p_shape[0]
                l = small.tile(p_shape, F32, tag=f"l{rows}", name=f"l{rows}")
                nc.vector.tensor_scalar(l[:], g_ps[:], scale_pp[:], None, op0=OP.mult)
                nc.vector.tensor_tensor(l[:], l[:], colb[:], op=OP.mult)
                nc.vector.tensor_tensor(l[:], l[:], maskb[:], op=OP.add)
                p = small.tile(p_shape, F32, tag=f"p{rows}", name=f"p{rows}")
                ssum = small.tile([rows, 1], F32, tag=f"ss{rows}", name=f"ss{rows}")
                nc.scalar.activation(p[:], l[:], AF.Exp, accum_out=ssum[:])
                rsum = small.tile([rows, 1], F32, tag=f"rs{rows}", name=f"rs{rows}")
                nc.vector.reciprocal(rsum[:], ssum[:])
                att = small.tile(p_shape, out_dt, tag=f"att{rows}", name=f"att{rows}")
                nc.vector.tensor_scalar(att[:], p[:], rsum[:], None, op0=OP.mult)
                return att

            A1 = softmax(g1, scale1, rab, mask1, [C, 64], F32)  # fp32 for PE transpose
            A2 = softmax(g2, scale2, rkb, mask2, [64, 128], BF)

            # ---- W = wout @ A1 @ A2  (compute W^T directly) ----
            a1t_ps = ps_sm.tile([64, 128], F32, tag="sm")
            nc.tensor.transpose(a1t_ps[:], A1[:], ident[:])
            A1T = small.tile([64, 128], BF, tag="A1T")
            nc.vector.tensor_copy(A1T[:], a1t_ps[:])
            t1_ps = ps_sm.tile([C, 128], F32, tag="sm")
            nc.tensor.matmul(t1_ps[:], A1T[:], A2[:], start=True, stop=True)
            T1 = small.tile([C, 128], BF, tag="T1")
            nc.vector.tensor_copy(T1[:], t1_ps[:])
            wb_ps = ps_sm.tile([C, 128], F32, tag="sm")
            nc.tensor.matmul(wb_ps[:], T1[:], woutT[:], start=True, stop=True)
            Wb = small.tile([C, 128], BF, tag="Wb")
            nc.vector.tensor_copy(Wb[:], wb_ps[:])

            # ---- staggered pipeline: remaining v tiles + final MMs + out DMAs ----
            ogs = [big.tile([C, 2048], BF, tag=f"og{g}", name=f"og{g}")
                   for g in range(8)]

            def f_tile(i):
                g, ii = i // 4, i % 4
                fp = ps_sm.tile([C, N_TILE], F32, tag="sm", name=f"fp{i}")
                nc.tensor.matmul(
                    fp[:], Wb[:], v_sb[:, ts(i, N_TILE)], start=True, stop=True
                )
                if i % 6 == 5:
                    nc.vector.tensor_copy(ogs[g][:, ts(ii, N_TILE)], fp[:])
                else:
                    nc.scalar.copy(ogs[g][:, ts(ii, N_TILE)], fp[:])
                if ii == 3:
                    nc.sync.dma_start(out_d[:, ts(g, 2048)], ogs[g][:])

            for i in range(32):
                v_tile(i)
                if i >= 8:
                    f_tile(i - 8)
            for i in range(24, 32):
                f_tile(i)

    nc.compile()
    return nc


def _split_cols(arr):
    """even-cols-first / odd-cols-second along the last axis"""
    return np.concatenate([arr[..., 0::2], arr[..., 1::2]], axis=-1)


def _prep_inputs(x, illu_feat, wq, wk, wa_dw, wa_pw, wv, wout, temp_a, temp_v):
    xp = np.zeros((B, C, HP, HP), np.float32)
    xp[:, :, 1:-1, 1:-1] = x
    xp = _split_cols(xp).reshape(B, C, HP * HP).astype(BF16)
    il = illu_feat[:, :, :, WLIST].reshape(B, C, S_FULL).astype(BF16)

    wkT = np.empty((C, 9, 128), np.float32)
    waT = np.empty((C, 9, 64), np.float32)
    for t, (dy, dx) in enumerate(TAPS):
        wkT[:, t, :] = wk[:, :, dy, dx].T
        # fused dw+pw for a: W_t[d, c] = wa_pw[d, c] * wa_dw[c, t]; lhsT = (c, d)
        waT[:, t, :] = (wa_pw[:, :, 0, 0] * wa_dw[None, :, 0, dy, dx]).T
    wq9 = wq[:, 0, :, :].reshape(C, 9).astype(np.float32)

    heads_c = np.arange(C) // 16
    heads_d = np.arange(64) // 8
    mask1 = np.where(heads_d[None, :] == heads_c[:, None], 0.0, NEG).astype(np.float32)
    mask2 = np.where(heads_c[None, :] == heads_d[:, None], 0.0, NEG).astype(np.float32)

    consts = {
        "wkT": wkT.reshape(C, 9 * 128).astype(BF16),
        "waT": waT.reshape(C, 9 * 64).astype(BF16),
        "wvT": np.ascontiguousarray(wv[:, :, 0, 0].T).astype(BF16),
        "woutT": np.ascontiguousarray(wout[:, :, 0, 0].T).astype(BF16),
        "wq9": wq9,
        "ta": np.repeat(temp_a.ravel(), 16).reshape(C, 1).astype(np.float32),
        "tv": np.repeat(temp_v.ravel(), 8).reshape(64, 1).astype(np.float32),
        "mask1": mask1,
        "mask2": mask2,
        "ident": np.eye(128, dtype=np.float32),
        "ones": np.ones((1, 128), BF16),
    }
    return [dict(consts, xpad=xp[b], illu=il[b]) for b in range(B)]


LAST_RESULTS = None


def kernel(x, illu_feat, wq, wk, wa_dw, wa_pw, wv, wout, temp_a, temp_v):
    global LAST_RESULTS
    reps = int(_CACHE.get("reps", 1))
    loop_n = _CACHE.get("loop_n")
    key = f"nc{reps}_{loop_n}"
    if key not in _CACHE:
        _CACHE[key] = _build(reps, loop_n=loop_n)
    nc = _CACHE[key]

    in_maps = _prep_inputs(
        np.asarray(x, np.float32), np.asarray(illu_feat, np.float32),
        np.asarray(wq, np.float32), np.asarray(wk, np.float32),
        np.asarray(wa_dw, np.float32), np.asarray(wa_pw, np.float32),
        np.asarray(wv, np.float32), np.asarray(wout, np.float32),
        np.asarray(temp_a, np.float32), np.asarray(temp_v, np.float32),
    )

    from concourse.bass_utils import run_bass_kernel_spmd

    res = run_bass_kernel_spmd(nc, in_maps, core_ids=list(range(B)))
    LAST_RESULTS = res
    out = np.stack([np.asarray(res.results[b]["out"], np.float32) for b in range(B)])
    out = out.reshape(B, C, H, W)
    inv = np.empty_like(out)
    inv[:, :, :, WLIST] = out
    return inv

